# revision 22
# baseline (speedup 1.0000x reference)
"""Trainium2 Bass kernel for GNN copy_src -> segment-mean (dst-sharded, 8 cores).

Strategy
--------
- Partition dst nodes (and their incoming edges) across 8 NeuronCores:
  core c owns dst rows [c*6250, (c+1)*6250).
- Host-side "inspector" pass (numpy): bucket each core's edges by
  128-slot dst block, split each block's edges into two runs by src range
  (dma_gather indices are int16, so the 50000-row table is addressed as
  two halves), pad each run to a multiple of 128.  Padding is -1 where
  possible (the gather ucode trims trailing negative indices before
  generating descriptors, so padding costs no Q7 time); the first few
  calls use index 0 instead so their SBUF tiles get fully written before
  any stale data could be read.  Per-dst inverse degrees are computed
  host-side (pure index data) and shipped as an input.
- Device kernel (identical SPMD program on all 8 cores), all-bf16 data
  path with fp32 PSUM accumulation:
  * one dma_gather per (dst-block, table-half) segment pulls that
    segment's bf16 source rows (256 B each) from HBM into SBUF.  Gathers
    are spread over all 4 SWDGE queues (the ucode routes queue q to Q7
    core pair q), so up to 4 calls generate descriptors concurrently.
  * per gather call, build the 0/1 edge->slot matrices H for all its
    groups in ONE DVE op via is_equal(slot_value, iota) with broadcast
    APs; padded edges give all-zero rows.
  * TensorE bf16 matmuls accumulate H^T @ G (feature sums) in PSUM per
    128-slot block.
  * per block: multiply by the precomputed 1/deg column, DMA the
    [128, 128] result tile to the output shard.
- Host gathers the 8 output shards into the full [50000, 128] output.
"""

import os
import sys

import numpy as np

for _p in ("/opt/trn_rl_repo",):
    if os.path.isdir(_p) and _p not in sys.path:
        sys.path.insert(0, _p)

from concourse import bacc, mybir  # noqa: E402
import concourse.bass as bass  # noqa: E402
import concourse.tile as tile  # noqa: E402
from concourse.bass_utils import run_bass_kernel_spmd  # noqa: E402

N_NODES = 50000
N_EDGES = 600000
D_FEAT = 128
N_CORES = 8
NLOC = N_NODES // N_CORES          # 6250 dst nodes per core
BLK = 128                          # dst slots per PSUM block
NB = (NLOC + BLK - 1) // BLK       # 49 blocks per core
HALF = 32768                       # int16 index limit for dma_gather
SWDGE_SCRATCH = 16384              # SWDGE descriptor ring: bytes/partition
NUM_QUEUES = 4                     # gather ucode: queue q -> Q7 core pair q
GBUFS = 6                          # gather/h tile pool depth
MAXG = 16                          # max groups per gather call (ring capacity)
NCHUNK = 4                         # idx table load split (startup overlap)
# -1 trailing padding is trimmed by the gather ucode, but with a static
# num_idxs_reg the decode-side ring bookkeeping (descs reserved from the reg
# value) desyncs from the Q7's actual pushed count -> device-wedging DMA
# corruption (observed on HW; CoreSim's reg==valid-count assert is the same
# protocol).  Keep False unless num_idxs_reg is a per-core runtime register.
PAD_NEG = False

_cache = {}


def _segments(g):
    """Call list: one dma_gather per (block, half) segment, split at MAXG.

    Returns list of dicts with keys: lst, b, g0 (group offset within its
    list), ncg, first (True for the first sub-call of the segment; the
    trailing -1 trim only applies to the last sub-call = segment tail).
    """
    aoff = np.concatenate([[0], np.cumsum(g[:, 0])])
    boff = np.concatenate([[0], np.cumsum(g[:, 1])])
    calls = []
    for b in range(g.shape[0]):
        for lst, off in ((0, aoff), (1, boff)):
            total = int(g[b, lst])
            s = 0
            while s < total:
                n = min(MAXG, total - s)
                calls.append(dict(lst=lst, b=b, g0=int(off[b]) + s, ncg=n,
                                  tail=(s + n == total)))
                s += n
    return calls


def _prepare(src, dst):
    """Inspector pass: group/pad edges per (core, block, src-half)."""
    core = dst // NLOC
    slot = dst % NLOC
    blk = slot // BLK
    srel = (slot % BLK).astype(np.float32)
    half = (src >= HALF).astype(np.int64)

    cnt = np.zeros((N_CORES, NB, 2), dtype=np.int64)
    np.add.at(cnt, (core, blk, half), 1)
    # groups per (block, half): shared across cores so the SPMD program is identical
    g = (cnt + 127) // 128
    g = g.max(axis=0)  # [NB, 2]
    zero_blocks = g.sum(axis=1) == 0
    g[zero_blocks, 0] = 1

    aoff = np.concatenate([[0], np.cumsum(g[:, 0])])
    boff = np.concatenate([[0], np.cumsum(g[:, 1])])
    GA, GB = int(aoff[-1]), int(boff[-1])
    G = GA + GB

    # -1 padding goes at each segment tail (the gather ucode trims trailing
    # negatives); the gather-tile buffers are memset once at startup so
    # trimmed lanes read zeros, never uninitialized SBUF
    calls = _segments(g)
    neg_ok = np.zeros((NB, 2), dtype=bool)
    for c in calls:
        if c["tail"] and PAD_NEG:
            neg_ok[c["b"], c["lst"]] = True

    key = ((core * NB + blk) * 2 + half)
    order = np.argsort(key, kind="stable")
    key_sorted = key[order]
    src_sorted = src[order]
    srel_sorted = srel[order]
    seg_starts = np.searchsorted(key_sorted, np.arange(N_CORES * NB * 2))
    seg_ends = np.searchsorted(key_sorted, np.arange(N_CORES * NB * 2), side="right")

    idx_vals = np.zeros((N_CORES, G, 128), dtype=np.int16)
    slot_vals = np.full((N_CORES, G, 128), -1.0, dtype=np.float32)
    for c in range(N_CORES):
        for b in range(NB):
            for h in range(2):
                s, e = seg_starts[(c * NB + b) * 2 + h], seg_ends[(c * NB + b) * 2 + h]
                n = e - s
                ng = int(g[b, h])
                if ng == 0:
                    continue
                g0 = (aoff[b] if h == 0 else GA + boff[b])
                iv = idx_vals[c, g0:g0 + ng].reshape(-1)
                sv = slot_vals[c, g0:g0 + ng].reshape(-1)
                if n:
                    sseg = src_sorted[s:e]
                    iv[:n] = (sseg - HALF * h).astype(np.int16)
                    sv[:n] = srel_sorted[s:e]
                if neg_ok[b, h]:
                    iv[n:] = -1

    # wrapped int16 layout for dma_gather: value (g, q) -> [q%16, 8*g + q//16],
    # replicated across the 8 sixteen-partition stripes
    w = idx_vals.reshape(N_CORES, G, 8, 16).transpose(0, 3, 1, 2).reshape(N_CORES, 16, G * 8)
    idxw = np.tile(w, (1, 8, 1))                       # [C, 128, G*8] int16
    slotw = slot_vals.transpose(0, 2, 1).copy()        # [C, 128, G] f32

    # per-core inverse degrees, [C, 128, NB]: invd[c, p, b] = 1/max(deg, 1)
    deg = np.bincount(dst, minlength=N_NODES).astype(np.float64)
    invd = (1.0 / np.maximum(deg, 1.0)).astype(np.float32)
    invdw = np.zeros((N_CORES, 128, NB), dtype=np.float32)
    for c in range(N_CORES):
        block = np.zeros(NB * BLK, dtype=np.float32)
        block[:NLOC] = invd[c * NLOC:(c + 1) * NLOC]
        invdw[c] = block.reshape(NB, BLK).T

    layout = dict(g=g, aoff=aoff, boff=boff, GA=GA, GB=GB)
    return idxw, slotw, invdw, layout


def _build_program(layout):
    g, aoff, boff = layout["g"], layout["aoff"], layout["boff"]
    GA, GB = layout["GA"], layout["GB"]
    G = GA + GB
    f32 = mybir.dt.float32
    bf16 = mybir.dt.bfloat16

    nc = bacc.Bacc("TRN2", target_bir_lowering=False, debug=False,
                   num_devices=N_CORES, dynamic_dma_scratch_size=SWDGE_SCRATCH,
                   num_swdge_queues=NUM_QUEUES)
    # two separate tensors: dma_gather's ucode mishandles nonzero source-AP
    # offsets on HW, so each int16-addressable half gets its own tensor
    embA = nc.dram_tensor("embA", [HALF, D_FEAT], bf16, kind="ExternalInput").ap()
    embB = nc.dram_tensor("embB", [N_NODES - HALF, D_FEAT], bf16, kind="ExternalInput").ap()
    # H is built with 1-input DVE ops (tensor_scalar is_equal against a
    # per-partition scalar): any 2-input DVE op streams via the second SBUF
    # read port, which is the port shared with GpSimd -- a lock that starves
    # SWDGE descriptor generation (the kernel's critical path).  bf16 at
    # 2 elem/cycle is exactly one port's width, so these ops never touch it.
    iota = nc.dram_tensor("iota", [128, BLK], bf16, kind="ExternalInput").ap()
    idxw = nc.dram_tensor("idxw", [128, G * 8], mybir.dt.int16, kind="ExternalInput").ap()
    slotw = nc.dram_tensor("slotw", [128, G], f32, kind="ExternalInput").ap()
    invdw = nc.dram_tensor("invdw", [128, NB], f32, kind="ExternalInput").ap()
    out = nc.dram_tensor("out", [NLOC, D_FEAT], f32, kind="ExternalOutput").ap()

    calls = _segments(g)
    maxg = max(c["ncg"] for c in calls)
    # column ranges in idxw per call; bucket calls into NCHUNK idx tiles
    # (split at call boundaries) so early gathers start before the whole
    # index table has loaded
    cols = []
    for c in calls:
        scol0 = c["g0"] if c["lst"] == 0 else GA + c["g0"]
        cols.append((scol0 * 8, c["ncg"] * 8, scol0))
    total_cols = G * 8
    target = (total_cols + NCHUNK - 1) // NCHUNK
    # calls are not column-ordered (A and B interleave); chunk by column space
    # instead: chunk k covers columns [k*target, (k+1)*target), and each call
    # is assigned to the chunk containing its first column; chunk tiles
    # overlap-load enough columns to cover calls that straddle a boundary.
    chunk_lo = [min(k * target, total_cols) for k in range(NCHUNK)]
    chunk_hi = [min((k + 1) * target, total_cols) for k in range(NCHUNK)]
    call_chunk = []
    for (c0, ncols, _s) in cols:
        k = min(c0 // target, NCHUNK - 1)
        call_chunk.append(k)
        chunk_hi[k] = max(chunk_hi[k], c0 + ncols)

    with tile.TileContext(nc) as tc:
        with (
            tc.tile_pool(name="const", bufs=1) as cpool,
            tc.tile_pool(name="gath", bufs=GBUFS) as gpool,
            tc.tile_pool(name="hbuf", bufs=3 * GBUFS) as hpool,
            tc.tile_pool(name="evict", bufs=4) as epool,
            tc.tile_pool(name="psum", bufs=4, space="PSUM") as ppool,
        ):
            idx_tiles = []
            for k in range(NCHUNK):
                w = chunk_hi[k] - chunk_lo[k]
                t = cpool.tile([128, w], mybir.dt.int16, tag=f"idx{k}")
                nc.sync.dma_start(out=t[:], in_=idxw[:, chunk_lo[k]:chunk_hi[k]])
                idx_tiles.append(t)
                if k == 0:
                    # small constants right after the first idx chunk
                    iota_sb = cpool.tile([128, BLK], bf16, tag="iota")
                    nc.sync.dma_start(out=iota_sb[:], in_=iota[:])
                    slot_sb = cpool.tile([128, G], f32, tag="slot")
                    nc.sync.dma_start(out=slot_sb[:], in_=slotw[:])
                    invd_sb = cpool.tile([128, NB], f32, tag="invd")
                    nc.sync.dma_start(out=invd_sb[:], in_=invdw[:])

            srcs = {0: embA, 1: embB}
            # issue order: one gather + one h-build per call, block-major
            call_of = {}
            for k, c in enumerate(calls):
                call_of.setdefault((c["lst"], c["b"]), []).append(k)
            tiles = [None] * len(calls)

            def issue_call(k):
                c = calls[k]
                c0, ncols, scol0 = cols[k]
                ncg = c["ncg"]
                ck = call_chunk[k]
                it = idx_tiles[ck]
                t = gpool.tile([128, maxg * 128], bf16, tag="g")
                nc.gpsimd.dma_gather(
                    out_ap=t[:, :ncg * 128].rearrange("p (n e) -> p n e", e=128),
                    in_ap=srcs[c["lst"]],
                    idxs_ap=it[:, c0 - chunk_lo[ck]:c0 - chunk_lo[ck] + ncols],
                    num_idxs=ncg * 128,
                    num_idxs_reg=ncg * 128,
                    elem_size=D_FEAT,
                    single_packet=(ncg <= 8),
                )
                tiles[k] = t

            for b in range(NB):
                bcalls = call_of.get((0, b), []) + call_of.get((1, b), [])
                for k in bcalls:
                    issue_call(k)
                psum_s = ppool.tile([128, BLK], f32, tag="ps")
                ngrp = sum(calls[k]["ncg"] for k in bcalls)
                i = 0
                for k in bcalls:
                    t = tiles[k]
                    scol0 = cols[k][2]
                    for j in range(calls[k]["ncg"]):
                        # 1-input build: is_equal(iota_row, slot_val[partition])
                        # -- single SBUF read port, no GpSimd port contention
                        h = hpool.tile([128, BLK], bf16, tag="h")
                        nc.vector.tensor_scalar(
                            out=h[:], in0=iota_sb[:],
                            scalar1=slot_sb[:, scol0 + j:scol0 + j + 1],
                            scalar2=None, op0=mybir.AluOpType.is_equal,
                        )
                        nc.tensor.matmul(
                            out=psum_s[:],
                            lhsT=h[:],
                            rhs=t[:, j * 128:(j + 1) * 128],
                            start=(i == 0), stop=(i == ngrp - 1),
                        )
                        i += 1
                    tiles[k] = None
                ot = epool.tile([128, BLK], f32, tag="ot")
                nc.scalar.activation(
                    out=ot[:], in_=psum_s[:],
                    func=mybir.ActivationFunctionType.Copy,
                    scale=invd_sb[:, b:b + 1],
                )
                rows = min(BLK, NLOC - b * BLK)
                nc.sync.dma_start(out=out[b * BLK:b * BLK + rows, :],
                                  in_=ot[:rows, :])

    # Tile's scheduling pass reorders instructions and round-robins SWDGE
    # completion sems over 8 DMASW lanes in FINAL order.  A sem may only ever
    # be incremented from one SWDGE queue (ring-reclaim correctness), so the
    # queue must be a function of the assigned lane: queue = lane % NUM_QUEUES.
    if NUM_QUEUES > 1:
        from concourse.tile_scheduler import PROC_NAME_TO_IDX
        lane_of = {PROC_NAME_TO_IDX[f"DMASW{i}"]: i for i in range(8)}
        fn = nc.m.functions[0]
        insts = [i for blk_ in fn.blocks for i in blk_.instructions]
        for inst in insts:
            if isinstance(inst, mybir.InstDMAGatherAnt):
                lane = lane_of.get(inst.bass_scheduled_proc)
                assert lane is not None, "gather not on a DMASW lane"
                inst.queue_num = lane % NUM_QUEUES

    nc.compile()
    return nc


def _in_maps(author_emb, src, dst):
    emb = np.ascontiguousarray(np.asarray(author_emb, dtype=np.float32))
    src = np.asarray(src).astype(np.int64)
    dst = np.asarray(dst).astype(np.int64)
    assert emb.shape == (N_NODES, D_FEAT) and src.shape == (N_EDGES,)

    idxw, slotw, invdw, layout = _prepare(src, dst)
    key = (layout["GA"], layout["GB"], layout["g"].tobytes())
    if key not in _cache:
        _cache[key] = _build_program(layout)
    nc = _cache[key]

    import ml_dtypes
    embh = emb.astype(ml_dtypes.bfloat16)
    iota_np = np.broadcast_to(np.arange(BLK, dtype=np.float32), (128, BLK)).astype(ml_dtypes.bfloat16)
    embA = np.ascontiguousarray(embh[:HALF])
    embB = np.ascontiguousarray(embh[HALF:])
    maps = [
        {"embA": embA, "embB": embB, "iota": np.ascontiguousarray(iota_np),
         "idxw": idxw[c], "slotw": slotw[c],
         "invdw": invdw[c]}
        for c in range(N_CORES)
    ]
    return nc, maps


def kernel(author_emb, src, dst, n_nodes):
    nc, maps = _in_maps(author_emb, src, dst)
    res = run_bass_kernel_spmd(nc, maps, list(range(N_CORES)))
    out = np.empty((N_NODES, D_FEAT), dtype=np.float32)
    for c in range(N_CORES):
        out[c * NLOC:(c + 1) * NLOC] = res.results[c]["out"]
    return out


# revision 27
# speedup vs baseline: 2.0807x; 2.0807x over previous
"""Trainium2 Bass kernel for GNN copy_src -> segment-mean (dst-sharded, 8 cores).

Strategy
--------
- Partition dst nodes (and their incoming edges) across 8 NeuronCores:
  core c owns dst rows [c*6250, (c+1)*6250).
- Host-side "inspector" pass (numpy): bucket each core's edges by
  128-slot dst block, split each block's edges into two runs by src range
  (dma_gather indices are int16, so the 50000-row table is addressed as
  two halves), pad each run to a multiple of 128.  Padding is -1 where
  possible (the gather ucode trims trailing negative indices before
  generating descriptors, so padding costs no Q7 time); the first few
  calls use index 0 instead so their SBUF tiles get fully written before
  any stale data could be read.  Per-dst inverse degrees are computed
  host-side (pure index data) and shipped as an input.
- Device kernel (identical SPMD program on all 8 cores), all-bf16 data
  path with fp32 PSUM accumulation:
  * one dma_gather per (dst-block, table-half) segment pulls that
    segment's bf16 source rows (256 B each) from HBM into SBUF.  Gathers
    are spread over all 4 SWDGE queues (the ucode routes queue q to Q7
    core pair q), so up to 4 calls generate descriptors concurrently.
  * per gather call, build the 0/1 edge->slot matrices H for all its
    groups in ONE DVE op via is_equal(slot_value, iota) with broadcast
    APs; padded edges give all-zero rows.
  * TensorE bf16 matmuls accumulate H^T @ G (feature sums) in PSUM per
    128-slot block.
  * per block: multiply by the precomputed 1/deg column, DMA the
    [128, 128] result tile to the output shard.
- Host gathers the 8 output shards into the full [50000, 128] output.
"""

import os
import sys

import numpy as np

for _p in ("/opt/trn_rl_repo",):
    if os.path.isdir(_p) and _p not in sys.path:
        sys.path.insert(0, _p)

from concourse import bacc, mybir  # noqa: E402
import concourse.bass as bass  # noqa: E402
import concourse.tile as tile  # noqa: E402
from concourse.bass_utils import run_bass_kernel_spmd  # noqa: E402

N_NODES = 50000
N_EDGES = 600000
D_FEAT = 128
N_CORES = 8
NLOC = N_NODES // N_CORES          # 6250 dst nodes per core
BLK = 128                          # dst slots per PSUM block
NB = (NLOC + BLK - 1) // BLK       # 49 blocks per core
HALF = 32768                       # int16 index limit for dma_gather
SWDGE_SCRATCH = 16384              # SWDGE descriptor ring: bytes/partition
NUM_QUEUES = 4                     # gather ucode: queue q -> Q7 core pair q
GBUFS = 6                          # gather/h tile pool depth
MAXG = 16                          # max groups per gather call (ring capacity)
NCHUNK = 4                         # idx table load split (startup overlap)
# -1 trailing padding is trimmed by the gather ucode, but with a static
# num_idxs_reg the decode-side ring bookkeeping (descs reserved from the reg
# value) desyncs from the Q7's actual pushed count -> device-wedging DMA
# corruption (observed on HW; CoreSim's reg==valid-count assert is the same
# protocol).  Keep False unless num_idxs_reg is a per-core runtime register.
PAD_NEG = False

_cache = {}


def _segments(g):
    """Call list: one dma_gather per (block, half) segment, split at MAXG.

    Returns list of dicts with keys: lst, b, g0 (group offset within its
    list), ncg, first (True for the first sub-call of the segment; the
    trailing -1 trim only applies to the last sub-call = segment tail).
    """
    aoff = np.concatenate([[0], np.cumsum(g[:, 0])])
    boff = np.concatenate([[0], np.cumsum(g[:, 1])])
    calls = []
    for b in range(g.shape[0]):
        for lst, off in ((0, aoff), (1, boff)):
            total = int(g[b, lst])
            s = 0
            while s < total:
                n = min(MAXG, total - s)
                calls.append(dict(lst=lst, b=b, g0=int(off[b]) + s, ncg=n,
                                  tail=(s + n == total)))
                s += n
    return calls


def _balance_core(dA, dB, gA, gB):
    """Assign one core's nodes to blocks, packing per-half edge counts under
    each block's group capacity.  Returns slot id per node, or None."""
    remA = gA.astype(np.int64) * BLK
    remB = gB.astype(np.int64) * BLK
    remN = np.full(NB, BLK, dtype=np.int64)
    order = np.argsort(-(dA * 2 + dB), kind="stable")
    blk_of = np.empty(dA.shape[0], np.int64)
    for n in order:
        ok = (remA >= dA[n]) & (remB >= dB[n]) & (remN > 0)
        if not ok.any():
            return None
        # maximize the tightest remaining margin (caps are lumpy, so best-fit
        # toward caps, not equal loads); node room as a light tiebreak
        mA = (remA - dA[n]) * 2
        mB = (remB - dB[n]) * 4
        score = np.where(ok, np.minimum(mA, mB) + remN * 8, -(10 ** 9))
        b = int(np.argmax(score))
        blk_of[n] = b
        remA[b] -= dA[n]
        remB[b] -= dB[n]
        remN[b] -= 1
    slot = np.empty(dA.shape[0], np.int64)
    for b in range(NB):
        nodes = np.where(blk_of == b)[0]
        slot[nodes] = b * BLK + np.arange(len(nodes))
    return slot


def _prepare(src, dst):
    """Inspector pass.

    The dst->slot mapping within each core is ours to choose, so a host-side
    bin-packing assigns nodes to 128-slot blocks such that every block's
    per-half edge count packs nearly exactly into 128-edge groups -- this
    removes the padding that a fixed dst-order layout pays (both the
    round-up per block and the max-over-cores slack).
    """
    core = dst // NLOC
    half = (src >= HALF).astype(np.int64)

    # per-node per-half degrees
    degA = np.bincount(dst[half == 0], minlength=N_NODES)
    degB = np.bincount(dst[half == 1], minlength=N_NODES)
    Acnt = degA.reshape(N_CORES, NLOC).sum(axis=1)
    Bcnt = degB.reshape(N_CORES, NLOC).sum(axis=1)

    GAL = int(np.ceil(Acnt.max() / BLK)) + 3
    GBL = int(np.ceil(Bcnt.max() / BLK)) + 3
    slots = None
    for _attempt in range(6):
        gA = np.full(NB, GAL // NB, dtype=np.int64)
        gA[:GAL % NB] += 1
        gB = np.full(NB, GBL // NB, dtype=np.int64)
        gB[:GBL % NB] += 1
        trial = []
        for c in range(N_CORES):
            s = _balance_core(degA[c * NLOC:(c + 1) * NLOC],
                              degB[c * NLOC:(c + 1) * NLOC], gA, gB)
            if s is None:
                break
            trial.append(s)
        if len(trial) == N_CORES:
            slots = trial
            break
        GAL += 2
        GBL += 2
    if slots is None:
        # fall back to the identity layout (node i -> slot i)
        slots = [np.arange(NLOC, dtype=np.int64) for _ in range(N_CORES)]

    # g from the REALIZED counts (feasible for balanced and fallback alike)
    slot_all = np.concatenate(slots)
    eb = slot_all[dst] // BLK
    cnt = np.zeros((N_CORES, NB, 2), dtype=np.int64)
    np.add.at(cnt, (core, eb, half), 1)
    g = (cnt + BLK - 1) // BLK
    g = g.max(axis=0)  # [NB, 2]
    g[(g.sum(axis=1) == 0), 0] = 1

    # per-edge block/srel under the balanced mapping
    # slots are per-core local; nodes are contiguous per core, so the
    # concatenation is indexable by global node id
    eslot = slot_all[dst]
    blk = eslot // BLK
    srel = (eslot % BLK).astype(np.float32)

    aoff = np.concatenate([[0], np.cumsum(g[:, 0])])
    boff = np.concatenate([[0], np.cumsum(g[:, 1])])
    GA, GB = int(aoff[-1]), int(boff[-1])
    G = GA + GB

    calls = _segments(g)
    neg_ok = np.zeros((NB, 2), dtype=bool)
    for c in calls:
        if c["tail"] and PAD_NEG:
            neg_ok[c["b"], c["lst"]] = True

    key = ((core * NB + blk) * 2 + half)
    order = np.argsort(key, kind="stable")
    key_sorted = key[order]
    src_sorted = src[order]
    srel_sorted = srel[order]
    seg_starts = np.searchsorted(key_sorted, np.arange(N_CORES * NB * 2))
    seg_ends = np.searchsorted(key_sorted, np.arange(N_CORES * NB * 2), side="right")

    idx_vals = np.zeros((N_CORES, G, 128), dtype=np.int16)
    slot_vals = np.full((N_CORES, G, 128), -1.0, dtype=np.float32)
    for c in range(N_CORES):
        for b in range(NB):
            for h in range(2):
                s, e = seg_starts[(c * NB + b) * 2 + h], seg_ends[(c * NB + b) * 2 + h]
                n = e - s
                ng = int(g[b, h])
                if ng == 0:
                    continue
                assert n <= ng * 128
                g0 = (aoff[b] if h == 0 else GA + boff[b])
                iv = idx_vals[c, g0:g0 + ng].reshape(-1)
                sv = slot_vals[c, g0:g0 + ng].reshape(-1)
                if n:
                    sseg = src_sorted[s:e]
                    iv[:n] = (sseg - HALF * h).astype(np.int16)
                    sv[:n] = srel_sorted[s:e]
                if neg_ok[b, h]:
                    iv[n:] = -1

    # wrapped int16 layout for dma_gather: value (g, q) -> [q%16, 8*g + q//16],
    # replicated across the 8 sixteen-partition stripes
    w = idx_vals.reshape(N_CORES, G, 8, 16).transpose(0, 3, 1, 2).reshape(N_CORES, 16, G * 8)
    idxw = np.tile(w, (1, 8, 1))                       # [C, 128, G*8] int16
    slotw = slot_vals.transpose(0, 2, 1).copy()        # [C, 128, G] f32

    # per-core inverse degrees at their balanced slots, [C, 128, NB]
    deg = (degA + degB).astype(np.float64)
    invd = (1.0 / np.maximum(deg, 1.0)).astype(np.float32)
    invdw = np.zeros((N_CORES, 128, NB), dtype=np.float32)
    for c in range(N_CORES):
        col = np.zeros(NB * BLK, dtype=np.float32)
        col[slots[c]] = invd[c * NLOC:(c + 1) * NLOC]
        invdw[c] = col.reshape(NB, BLK).T

    layout = dict(g=g, aoff=aoff, boff=boff, GA=GA, GB=GB)
    return idxw, slotw, invdw, slots, layout


def _build_program(layout):
    g, aoff, boff = layout["g"], layout["aoff"], layout["boff"]
    GA, GB = layout["GA"], layout["GB"]
    G = GA + GB
    f32 = mybir.dt.float32
    bf16 = mybir.dt.bfloat16

    nc = bacc.Bacc("TRN2", target_bir_lowering=False, debug=False,
                   num_devices=N_CORES, dynamic_dma_scratch_size=SWDGE_SCRATCH,
                   num_swdge_queues=NUM_QUEUES)
    # two separate tensors: dma_gather's ucode mishandles nonzero source-AP
    # offsets on HW, so each int16-addressable half gets its own tensor
    embA = nc.dram_tensor("embA", [HALF, D_FEAT], bf16, kind="ExternalInput").ap()
    embB = nc.dram_tensor("embB", [N_NODES - HALF, D_FEAT], bf16, kind="ExternalInput").ap()
    # H is built with 1-input DVE ops (tensor_scalar is_equal against a
    # per-partition scalar): any 2-input DVE op streams via the second SBUF
    # read port, which is the port shared with GpSimd -- a lock that starves
    # SWDGE descriptor generation (the kernel's critical path).  bf16 at
    # 2 elem/cycle is exactly one port's width, so these ops never touch it.
    iota = nc.dram_tensor("iota", [128, BLK], bf16, kind="ExternalInput").ap()
    idxw = nc.dram_tensor("idxw", [128, G * 8], mybir.dt.int16, kind="ExternalInput").ap()
    slotw = nc.dram_tensor("slotw", [128, G], bf16, kind="ExternalInput").ap()
    invdw = nc.dram_tensor("invdw", [128, NB], f32, kind="ExternalInput").ap()
    out = nc.dram_tensor("out", [NB * BLK, D_FEAT], f32, kind="ExternalOutput").ap()

    calls = _segments(g)
    maxg = max(c["ncg"] for c in calls)
    # column ranges in idxw per call; bucket calls into NCHUNK idx tiles
    # (split at call boundaries) so early gathers start before the whole
    # index table has loaded
    cols = []
    for c in calls:
        scol0 = c["g0"] if c["lst"] == 0 else GA + c["g0"]
        cols.append((scol0 * 8, c["ncg"] * 8, scol0))
    total_cols = G * 8
    target = (total_cols + NCHUNK - 1) // NCHUNK
    # calls are not column-ordered (A and B interleave); chunk by column space
    # instead: chunk k covers columns [k*target, (k+1)*target), and each call
    # is assigned to the chunk containing its first column; chunk tiles
    # overlap-load enough columns to cover calls that straddle a boundary.
    chunk_lo = [min(k * target, total_cols) for k in range(NCHUNK)]
    chunk_hi = [min((k + 1) * target, total_cols) for k in range(NCHUNK)]
    call_chunk = []
    for (c0, ncols, _s) in cols:
        k = min(c0 // target, NCHUNK - 1)
        call_chunk.append(k)
        chunk_hi[k] = max(chunk_hi[k], c0 + ncols)

    with tile.TileContext(nc) as tc:
        with (
            tc.tile_pool(name="const", bufs=1) as cpool,
            tc.tile_pool(name="gath", bufs=GBUFS) as gpool,
            tc.tile_pool(name="hbuf", bufs=GBUFS) as hpool,
            tc.tile_pool(name="evict", bufs=4) as epool,
            tc.tile_pool(name="psum", bufs=4, space="PSUM") as ppool,
        ):
            idx_tiles = []
            for k in range(NCHUNK):
                w = chunk_hi[k] - chunk_lo[k]
                t = cpool.tile([128, w], mybir.dt.int16, tag=f"idx{k}")
                nc.sync.dma_start(out=t[:], in_=idxw[:, chunk_lo[k]:chunk_hi[k]])
                idx_tiles.append(t)
                if k == 0:
                    # small constants right after the first idx chunk
                    iota_sb = cpool.tile([128, BLK], bf16, tag="iota")
                    nc.sync.dma_start(out=iota_sb[:], in_=iota[:])
                    slot_sb = cpool.tile([128, G], bf16, tag="slot")
                    nc.sync.dma_start(out=slot_sb[:], in_=slotw[:])
                    invd_sb = cpool.tile([128, NB], f32, tag="invd")
                    nc.sync.dma_start(out=invd_sb[:], in_=invdw[:])

            srcs = {0: embA, 1: embB}
            # issue order: one gather + one h-build per call, block-major
            call_of = {}
            for k, c in enumerate(calls):
                call_of.setdefault((c["lst"], c["b"]), []).append(k)
            tiles = [None] * len(calls)

            def issue_call(k):
                c = calls[k]
                c0, ncols, scol0 = cols[k]
                ncg = c["ncg"]
                ck = call_chunk[k]
                it = idx_tiles[ck]
                t = gpool.tile([128, maxg * 128], bf16, tag="g")
                nc.gpsimd.dma_gather(
                    out_ap=t[:, :ncg * 128].rearrange("p (n e) -> p n e", e=128),
                    in_ap=srcs[c["lst"]],
                    idxs_ap=it[:, c0 - chunk_lo[ck]:c0 - chunk_lo[ck] + ncols],
                    num_idxs=ncg * 128,
                    num_idxs_reg=ncg * 128,
                    elem_size=D_FEAT,
                    single_packet=(ncg <= 8),
                )
                # batched H build for all ncg groups of this call: one DVE op.
                # (2-input DVE ops lock the GpSimd-shared SBUF port while they
                # stream, so keep them bf16 -- 2 elem/cycle halves the lock
                # time -- and batched: per-group ops cost ~700ns dispatch each
                # on HW, which swamps their 67ns of streaming.)
                h = hpool.tile([128, maxg * BLK], bf16, tag="h")
                sv = slot_sb[:, scol0:scol0 + ncg]
                sv3 = sv.unsqueeze(2).to_broadcast([128, ncg, BLK])
                io3 = iota_sb[:].unsqueeze(1).to_broadcast([128, ncg, BLK])
                nc.vector.tensor_tensor(
                    out=h[:, :ncg * BLK].rearrange("p (n e) -> p n e", e=BLK),
                    in0=sv3,
                    in1=io3,
                    op=mybir.AluOpType.is_equal,
                )
                tiles[k] = (t, h)

            for b in range(NB):
                bcalls = call_of.get((0, b), []) + call_of.get((1, b), [])
                for k in bcalls:
                    issue_call(k)
                psum_s = ppool.tile([128, BLK], f32, tag="ps")
                ngrp = sum(calls[k]["ncg"] for k in bcalls)
                i = 0
                for k in bcalls:
                    t, h = tiles[k]
                    for j in range(calls[k]["ncg"]):
                        nc.tensor.matmul(
                            out=psum_s[:],
                            lhsT=h[:, j * BLK:(j + 1) * BLK],
                            rhs=t[:, j * 128:(j + 1) * 128],
                            start=(i == 0), stop=(i == ngrp - 1),
                        )
                        i += 1
                    tiles[k] = None
                ot = epool.tile([128, BLK], f32, tag="ot")
                nc.scalar.activation(
                    out=ot[:], in_=psum_s[:],
                    func=mybir.ActivationFunctionType.Copy,
                    scale=invd_sb[:, b:b + 1],
                )
                nc.sync.dma_start(out=out[b * BLK:(b + 1) * BLK, :],
                                  in_=ot[:, :])

    # Tile's scheduling pass reorders instructions and round-robins SWDGE
    # completion sems over 8 DMASW lanes in FINAL order.  A sem may only ever
    # be incremented from one SWDGE queue (ring-reclaim correctness), so the
    # queue must be a function of the assigned lane: queue = lane % NUM_QUEUES.
    if NUM_QUEUES > 1:
        from concourse.tile_scheduler import PROC_NAME_TO_IDX
        lane_of = {PROC_NAME_TO_IDX[f"DMASW{i}"]: i for i in range(8)}
        fn = nc.m.functions[0]
        insts = [i for blk_ in fn.blocks for i in blk_.instructions]
        for inst in insts:
            if isinstance(inst, mybir.InstDMAGatherAnt):
                lane = lane_of.get(inst.bass_scheduled_proc)
                assert lane is not None, "gather not on a DMASW lane"
                inst.queue_num = lane % NUM_QUEUES

    nc.compile()
    return nc


def _in_maps(author_emb, src, dst):
    emb = np.ascontiguousarray(np.asarray(author_emb, dtype=np.float32))
    src = np.asarray(src).astype(np.int64)
    dst = np.asarray(dst).astype(np.int64)
    assert emb.shape == (N_NODES, D_FEAT) and src.shape == (N_EDGES,)

    idxw, slotw, invdw, slots, layout = _prepare(src, dst)
    key = (layout["GA"], layout["GB"], layout["g"].tobytes())
    if key not in _cache:
        _cache[key] = _build_program(layout)
    nc = _cache[key]

    import ml_dtypes
    embh = emb.astype(ml_dtypes.bfloat16)
    iota_np = np.broadcast_to(np.arange(BLK, dtype=np.float32), (128, BLK)).astype(ml_dtypes.bfloat16)
    embA = np.ascontiguousarray(embh[:HALF])
    embB = np.ascontiguousarray(embh[HALF:])
    maps = [
        {"embA": embA, "embB": embB, "iota": np.ascontiguousarray(iota_np),
         "idxw": idxw[c], "slotw": slotw[c].astype(ml_dtypes.bfloat16),
         "invdw": invdw[c]}
        for c in range(N_CORES)
    ]
    return nc, maps, slots


def kernel(author_emb, src, dst, n_nodes):
    nc, maps, slots = _in_maps(author_emb, src, dst)
    res = run_bass_kernel_spmd(nc, maps, list(range(N_CORES)))
    out = np.empty((N_NODES, D_FEAT), dtype=np.float32)
    for c in range(N_CORES):
        out[c * NLOC:(c + 1) * NLOC] = res.results[c]["out"][slots[c]]
    return out


# revision 29
# speedup vs baseline: 2.1714x; 1.0436x over previous
"""Trainium2 Bass kernel for GNN copy_src -> segment-mean (dst-sharded, 8 cores).

Strategy
--------
- Partition dst nodes (and their incoming edges) across 8 NeuronCores:
  core c owns dst rows [c*6250, (c+1)*6250).
- Host-side "inspector" pass (numpy): each core's dst nodes are
  bin-packed into 128-slot blocks so that every block's per-src-half edge
  count packs nearly exactly into 128-edge gather groups (the dst->slot
  mapping is free to choose; the host inverts the permutation when
  unsharding).  Edges are bucketed per (block, src-half) -- dma_gather
  indices are int16, so the 50000-row table is addressed as two halves --
  and padded to a multiple of 128 with src-index-0 dummies whose slot
  sentinel -1 gives all-zero H rows.  Per-dst inverse degrees are
  computed host-side (pure index data) and shipped as an input.
- Device kernel (identical SPMD program on all 8 cores), all-bf16 data
  path with fp32 PSUM accumulation:
  * one dma_gather per (dst-block, table-half) segment pulls that
    segment's bf16 source rows (256 B each) from HBM into SBUF.  Gathers
    are spread over all 4 SWDGE queues (the ucode routes queue q to Q7
    core pair q), so up to 4 calls generate descriptors concurrently.
  * per gather call, build the 0/1 edge->slot matrices H for all its
    groups in ONE DVE op via is_equal(slot_value, iota) with broadcast
    APs; padded edges give all-zero rows.
  * TensorE bf16 matmuls accumulate H^T @ G (feature sums) in PSUM per
    128-slot block.
  * per block: multiply by the precomputed 1/deg column, DMA the
    [128, 128] result tile to the output shard.
- Host gathers the 8 output shards into the full [50000, 128] output.
"""

import os
import sys

import numpy as np

for _p in ("/opt/trn_rl_repo",):
    if os.path.isdir(_p) and _p not in sys.path:
        sys.path.insert(0, _p)

from concourse import bacc, mybir  # noqa: E402
import concourse.bass as bass  # noqa: E402
import concourse.tile as tile  # noqa: E402
from concourse.bass_utils import run_bass_kernel_spmd  # noqa: E402

N_NODES = 50000
N_EDGES = 600000
D_FEAT = 128
N_CORES = 8
NLOC = N_NODES // N_CORES          # 6250 dst nodes per core
BLK = 128                          # dst slots per PSUM block
NB = (NLOC + BLK - 1) // BLK       # 49 blocks per core
HALF = 32768                       # int16 index limit for dma_gather
SWDGE_SCRATCH = 16384              # SWDGE descriptor ring: bytes/partition
NUM_QUEUES = 4                     # gather ucode: queue q -> Q7 core pair q
GBUFS = 10                         # gather/h tile pool depth
MAXG = 16                          # max groups per gather call (ring capacity)
NCHUNK = 4                         # idx table load split (startup overlap)
# -1 trailing padding is trimmed by the gather ucode, but with a static
# num_idxs_reg the decode-side ring bookkeeping (descs reserved from the reg
# value) desyncs from the Q7's actual pushed count -> device-wedging DMA
# corruption (observed on HW; CoreSim's reg==valid-count assert is the same
# protocol).  Keep False unless num_idxs_reg is a per-core runtime register.
PAD_NEG = False

_cache = {}


def _segments(g):
    """Call list: one dma_gather per (block, half) segment, split at MAXG.

    Returns list of dicts with keys: lst, b, g0 (group offset within its
    list), ncg, first (True for the first sub-call of the segment; the
    trailing -1 trim only applies to the last sub-call = segment tail).
    """
    aoff = np.concatenate([[0], np.cumsum(g[:, 0])])
    boff = np.concatenate([[0], np.cumsum(g[:, 1])])
    calls = []
    for b in range(g.shape[0]):
        for lst, off in ((0, aoff), (1, boff)):
            total = int(g[b, lst])
            s = 0
            while s < total:
                n = min(MAXG, total - s)
                calls.append(dict(lst=lst, b=b, g0=int(off[b]) + s, ncg=n,
                                  tail=(s + n == total)))
                s += n
    return calls


def _balance_core(dA, dB, gA, gB):
    """Assign one core's nodes to blocks, packing per-half edge counts under
    each block's group capacity.  Returns slot id per node, or None."""
    remA = gA.astype(np.int64) * BLK
    remB = gB.astype(np.int64) * BLK
    remN = np.full(NB, BLK, dtype=np.int64)
    order = np.argsort(-(dA * 2 + dB), kind="stable")
    blk_of = np.empty(dA.shape[0], np.int64)
    for n in order:
        ok = (remA >= dA[n]) & (remB >= dB[n]) & (remN > 0)
        if not ok.any():
            return None
        # maximize the tightest remaining margin (caps are lumpy, so best-fit
        # toward caps, not equal loads); node room as a light tiebreak
        mA = (remA - dA[n]) * 2
        mB = (remB - dB[n]) * 4
        score = np.where(ok, np.minimum(mA, mB) + remN * 8, -(10 ** 9))
        b = int(np.argmax(score))
        blk_of[n] = b
        remA[b] -= dA[n]
        remB[b] -= dB[n]
        remN[b] -= 1
    slot = np.empty(dA.shape[0], np.int64)
    for b in range(NB):
        nodes = np.where(blk_of == b)[0]
        slot[nodes] = b * BLK + np.arange(len(nodes))
    return slot


def _prepare(src, dst):
    """Inspector pass.

    The dst->slot mapping within each core is ours to choose, so a host-side
    bin-packing assigns nodes to 128-slot blocks such that every block's
    per-half edge count packs nearly exactly into 128-edge groups -- this
    removes the padding that a fixed dst-order layout pays (both the
    round-up per block and the max-over-cores slack).
    """
    core = dst // NLOC
    half = (src >= HALF).astype(np.int64)

    # per-node per-half degrees
    degA = np.bincount(dst[half == 0], minlength=N_NODES)
    degB = np.bincount(dst[half == 1], minlength=N_NODES)
    Acnt = degA.reshape(N_CORES, NLOC).sum(axis=1)
    Bcnt = degB.reshape(N_CORES, NLOC).sum(axis=1)

    GAL = int(np.ceil(Acnt.max() / BLK)) + 3
    GBL = int(np.ceil(Bcnt.max() / BLK)) + 3
    slots = None
    for _attempt in range(6):
        gA = np.full(NB, GAL // NB, dtype=np.int64)
        gA[:GAL % NB] += 1
        gB = np.full(NB, GBL // NB, dtype=np.int64)
        gB[:GBL % NB] += 1
        trial = []
        for c in range(N_CORES):
            s = _balance_core(degA[c * NLOC:(c + 1) * NLOC],
                              degB[c * NLOC:(c + 1) * NLOC], gA, gB)
            if s is None:
                break
            trial.append(s)
        if len(trial) == N_CORES:
            slots = trial
            break
        GAL += 2
        GBL += 2
    if slots is None:
        # fall back to the identity layout (node i -> slot i)
        slots = [np.arange(NLOC, dtype=np.int64) for _ in range(N_CORES)]

    # g from the REALIZED counts (feasible for balanced and fallback alike)
    slot_all = np.concatenate(slots)
    eb = slot_all[dst] // BLK
    cnt = np.zeros((N_CORES, NB, 2), dtype=np.int64)
    np.add.at(cnt, (core, eb, half), 1)
    g = (cnt + BLK - 1) // BLK
    g = g.max(axis=0)  # [NB, 2]
    g[(g.sum(axis=1) == 0), 0] = 1

    # per-edge block/srel under the balanced mapping
    # slots are per-core local; nodes are contiguous per core, so the
    # concatenation is indexable by global node id
    eslot = slot_all[dst]
    blk = eslot // BLK
    srel = (eslot % BLK).astype(np.float32)

    aoff = np.concatenate([[0], np.cumsum(g[:, 0])])
    boff = np.concatenate([[0], np.cumsum(g[:, 1])])
    GA, GB = int(aoff[-1]), int(boff[-1])
    G = GA + GB

    calls = _segments(g)
    neg_ok = np.zeros((NB, 2), dtype=bool)
    for c in calls:
        if c["tail"] and PAD_NEG:
            neg_ok[c["b"], c["lst"]] = True

    key = ((core * NB + blk) * 2 + half)
    order = np.argsort(key, kind="stable")
    key_sorted = key[order]
    src_sorted = src[order]
    srel_sorted = srel[order]
    seg_starts = np.searchsorted(key_sorted, np.arange(N_CORES * NB * 2))
    seg_ends = np.searchsorted(key_sorted, np.arange(N_CORES * NB * 2), side="right")

    idx_vals = np.zeros((N_CORES, G, 128), dtype=np.int16)
    slot_vals = np.full((N_CORES, G, 128), -1.0, dtype=np.float32)
    for c in range(N_CORES):
        for b in range(NB):
            for h in range(2):
                s, e = seg_starts[(c * NB + b) * 2 + h], seg_ends[(c * NB + b) * 2 + h]
                n = e - s
                ng = int(g[b, h])
                if ng == 0:
                    continue
                assert n <= ng * 128
                g0 = (aoff[b] if h == 0 else GA + boff[b])
                iv = idx_vals[c, g0:g0 + ng].reshape(-1)
                sv = slot_vals[c, g0:g0 + ng].reshape(-1)
                if n:
                    sseg = src_sorted[s:e]
                    iv[:n] = (sseg - HALF * h).astype(np.int16)
                    sv[:n] = srel_sorted[s:e]
                if neg_ok[b, h]:
                    iv[n:] = -1

    # wrapped int16 layout for dma_gather: value (g, q) -> [q%16, 8*g + q//16],
    # replicated across the 8 sixteen-partition stripes
    w = idx_vals.reshape(N_CORES, G, 8, 16).transpose(0, 3, 1, 2).reshape(N_CORES, 16, G * 8)
    idxw = np.tile(w, (1, 8, 1))                       # [C, 128, G*8] int16
    slotw = slot_vals.transpose(0, 2, 1).copy()        # [C, 128, G] f32

    # per-core inverse degrees at their balanced slots, [C, 128, NB]
    deg = (degA + degB).astype(np.float64)
    invd = (1.0 / np.maximum(deg, 1.0)).astype(np.float32)
    invdw = np.zeros((N_CORES, 128, NB), dtype=np.float32)
    for c in range(N_CORES):
        col = np.zeros(NB * BLK, dtype=np.float32)
        col[slots[c]] = invd[c * NLOC:(c + 1) * NLOC]
        invdw[c] = col.reshape(NB, BLK).T

    layout = dict(g=g, aoff=aoff, boff=boff, GA=GA, GB=GB)
    return idxw, slotw, invdw, slots, layout


def _build_program(layout):
    g, aoff, boff = layout["g"], layout["aoff"], layout["boff"]
    GA, GB = layout["GA"], layout["GB"]
    G = GA + GB
    f32 = mybir.dt.float32
    bf16 = mybir.dt.bfloat16

    nc = bacc.Bacc("TRN2", target_bir_lowering=False, debug=False,
                   num_devices=N_CORES, dynamic_dma_scratch_size=SWDGE_SCRATCH,
                   num_swdge_queues=NUM_QUEUES)
    # two separate tensors: dma_gather's ucode mishandles nonzero source-AP
    # offsets on HW, so each int16-addressable half gets its own tensor
    embA = nc.dram_tensor("embA", [HALF, D_FEAT], bf16, kind="ExternalInput").ap()
    embB = nc.dram_tensor("embB", [N_NODES - HALF, D_FEAT], bf16, kind="ExternalInput").ap()
    # H is built with 1-input DVE ops (tensor_scalar is_equal against a
    # per-partition scalar): any 2-input DVE op streams via the second SBUF
    # read port, which is the port shared with GpSimd -- a lock that starves
    # SWDGE descriptor generation (the kernel's critical path).  bf16 at
    # 2 elem/cycle is exactly one port's width, so these ops never touch it.
    iota = nc.dram_tensor("iota", [128, BLK], bf16, kind="ExternalInput").ap()
    idxw = nc.dram_tensor("idxw", [128, G * 8], mybir.dt.int16, kind="ExternalInput").ap()
    slotw = nc.dram_tensor("slotw", [128, G], bf16, kind="ExternalInput").ap()
    invdw = nc.dram_tensor("invdw", [128, NB], f32, kind="ExternalInput").ap()
    out = nc.dram_tensor("out", [NB * BLK, D_FEAT], f32, kind="ExternalOutput").ap()

    calls = _segments(g)
    maxg = max(c["ncg"] for c in calls)
    # column ranges in idxw per call; bucket calls into NCHUNK idx tiles
    # (split at call boundaries) so early gathers start before the whole
    # index table has loaded
    cols = []
    for c in calls:
        scol0 = c["g0"] if c["lst"] == 0 else GA + c["g0"]
        cols.append((scol0 * 8, c["ncg"] * 8, scol0))
    total_cols = G * 8
    target = (total_cols + NCHUNK - 1) // NCHUNK
    first_cut = 128  # ~2 calls' worth of idx columns
    # calls are not column-ordered (A and B interleave); chunk by column space
    # instead: chunk k covers columns [k*target, (k+1)*target), and each call
    # is assigned to the chunk containing its first column; chunk tiles
    # overlap-load enough columns to cover calls that straddle a boundary.
    chunk_lo = [0] + [min(first_cut + k * target, total_cols) for k in range(NCHUNK - 1)]
    chunk_hi = [min(first_cut, total_cols)] + [min(first_cut + (k + 1) * target, total_cols) for k in range(NCHUNK - 1)]
    call_chunk = []
    for (c0, ncols, _s) in cols:
        if c0 < first_cut:
            k = 0
        else:
            k = min((c0 - first_cut) // target + 1, NCHUNK - 1)
        call_chunk.append(k)
        chunk_hi[k] = max(chunk_hi[k], c0 + ncols)

    with tile.TileContext(nc) as tc:
        with (
            tc.tile_pool(name="const", bufs=1) as cpool,
            tc.tile_pool(name="gath", bufs=GBUFS) as gpool,
            tc.tile_pool(name="hbuf", bufs=GBUFS) as hpool,
            tc.tile_pool(name="evict", bufs=6) as epool,
            tc.tile_pool(name="psum", bufs=6, space="PSUM") as ppool,
        ):
            idx_tiles = []
            for k in range(NCHUNK):
                w = chunk_hi[k] - chunk_lo[k]
                t = cpool.tile([128, w], mybir.dt.int16, tag=f"idx{k}")
                nc.sync.dma_start(out=t[:], in_=idxw[:, chunk_lo[k]:chunk_hi[k]])
                idx_tiles.append(t)
                if k == 0:
                    # small constants right after the first idx chunk
                    iota_sb = cpool.tile([128, BLK], bf16, tag="iota")
                    nc.sync.dma_start(out=iota_sb[:], in_=iota[:])
                    slot_sb = cpool.tile([128, G], bf16, tag="slot")
                    nc.sync.dma_start(out=slot_sb[:], in_=slotw[:])
                    invd_sb = cpool.tile([128, NB], f32, tag="invd")
                    nc.sync.dma_start(out=invd_sb[:], in_=invdw[:])

            srcs = {0: embA, 1: embB}
            # issue order: one gather + one h-build per call, block-major
            call_of = {}
            for k, c in enumerate(calls):
                call_of.setdefault((c["lst"], c["b"]), []).append(k)
            tiles = [None] * len(calls)

            def issue_call(k):
                c = calls[k]
                c0, ncols, scol0 = cols[k]
                ncg = c["ncg"]
                ck = call_chunk[k]
                it = idx_tiles[ck]
                t = gpool.tile([128, maxg * 128], bf16, tag="g")
                nc.gpsimd.dma_gather(
                    out_ap=t[:, :ncg * 128].rearrange("p (n e) -> p n e", e=128),
                    in_ap=srcs[c["lst"]],
                    idxs_ap=it[:, c0 - chunk_lo[ck]:c0 - chunk_lo[ck] + ncols],
                    num_idxs=ncg * 128,
                    num_idxs_reg=ncg * 128,
                    elem_size=D_FEAT,
                    single_packet=(ncg <= 8),
                )
                # batched H build for all ncg groups of this call: one DVE op.
                # (2-input DVE ops lock the GpSimd-shared SBUF port while they
                # stream, so keep them bf16 -- 2 elem/cycle halves the lock
                # time -- and batched: per-group ops cost ~700ns dispatch each
                # on HW, which swamps their 67ns of streaming.)
                h = hpool.tile([128, maxg * BLK], bf16, tag="h")
                sv = slot_sb[:, scol0:scol0 + ncg]
                sv3 = sv.unsqueeze(2).to_broadcast([128, ncg, BLK])
                io3 = iota_sb[:].unsqueeze(1).to_broadcast([128, ncg, BLK])
                nc.vector.tensor_tensor(
                    out=h[:, :ncg * BLK].rearrange("p (n e) -> p n e", e=BLK),
                    in0=sv3,
                    in1=io3,
                    op=mybir.AluOpType.is_equal,
                )
                tiles[k] = (t, h)

            for b in range(NB):
                bcalls = call_of.get((0, b), []) + call_of.get((1, b), [])
                for k in bcalls:
                    issue_call(k)
                psum_s = ppool.tile([128, BLK], f32, tag="ps")
                ngrp = sum(calls[k]["ncg"] for k in bcalls)
                i = 0
                for k in bcalls:
                    t, h = tiles[k]
                    for j in range(calls[k]["ncg"]):
                        nc.tensor.matmul(
                            out=psum_s[:],
                            lhsT=h[:, j * BLK:(j + 1) * BLK],
                            rhs=t[:, j * 128:(j + 1) * 128],
                            start=(i == 0), stop=(i == ngrp - 1),
                        )
                        i += 1
                    tiles[k] = None
                ot = epool.tile([128, BLK], f32, tag="ot")
                nc.scalar.activation(
                    out=ot[:], in_=psum_s[:],
                    func=mybir.ActivationFunctionType.Copy,
                    scale=invd_sb[:, b:b + 1],
                )
                nc.sync.dma_start(out=out[b * BLK:(b + 1) * BLK, :],
                                  in_=ot[:, :])

    # Tile's scheduling pass reorders instructions and round-robins SWDGE
    # completion sems over 8 DMASW lanes in FINAL order.  A sem may only ever
    # be incremented from one SWDGE queue (ring-reclaim correctness), so the
    # queue must be a function of the assigned lane: queue = lane % NUM_QUEUES.
    if NUM_QUEUES > 1:
        from concourse.tile_scheduler import PROC_NAME_TO_IDX
        lane_of = {PROC_NAME_TO_IDX[f"DMASW{i}"]: i for i in range(8)}
        fn = nc.m.functions[0]
        insts = [i for blk_ in fn.blocks for i in blk_.instructions]
        for inst in insts:
            if isinstance(inst, mybir.InstDMAGatherAnt):
                lane = lane_of.get(inst.bass_scheduled_proc)
                assert lane is not None, "gather not on a DMASW lane"
                inst.queue_num = lane % NUM_QUEUES

    nc.compile()
    return nc


def _in_maps(author_emb, src, dst):
    emb = np.ascontiguousarray(np.asarray(author_emb, dtype=np.float32))
    src = np.asarray(src).astype(np.int64)
    dst = np.asarray(dst).astype(np.int64)
    assert emb.shape == (N_NODES, D_FEAT) and src.shape == (N_EDGES,)

    idxw, slotw, invdw, slots, layout = _prepare(src, dst)
    key = (layout["GA"], layout["GB"], layout["g"].tobytes())
    if key not in _cache:
        _cache[key] = _build_program(layout)
    nc = _cache[key]

    import ml_dtypes
    embh = emb.astype(ml_dtypes.bfloat16)
    iota_np = np.broadcast_to(np.arange(BLK, dtype=np.float32), (128, BLK)).astype(ml_dtypes.bfloat16)
    embA = np.ascontiguousarray(embh[:HALF])
    embB = np.ascontiguousarray(embh[HALF:])
    maps = [
        {"embA": embA, "embB": embB, "iota": np.ascontiguousarray(iota_np),
         "idxw": idxw[c], "slotw": slotw[c].astype(ml_dtypes.bfloat16),
         "invdw": invdw[c]}
        for c in range(N_CORES)
    ]
    return nc, maps, slots


def kernel(author_emb, src, dst, n_nodes):
    nc, maps, slots = _in_maps(author_emb, src, dst)
    res = run_bass_kernel_spmd(nc, maps, list(range(N_CORES)))
    out = np.empty((N_NODES, D_FEAT), dtype=np.float32)
    for c in range(N_CORES):
        out[c * NLOC:(c + 1) * NLOC] = res.results[c]["out"][slots[c]]
    return out


# revision 30
# speedup vs baseline: 2.4183x; 1.1137x over previous
"""Trainium2 Bass kernel for GNN copy_src -> segment-mean (dst-sharded, 8 cores).

Strategy
--------
- Partition dst nodes (and their incoming edges) across 8 NeuronCores:
  core c owns dst rows [c*6250, (c+1)*6250).
- Host-side "inspector" pass (numpy): each core's dst nodes are
  bin-packed into 128-slot blocks so that every block's per-src-half edge
  count packs nearly exactly into 128-edge gather groups (the dst->slot
  mapping is free to choose; the host inverts the permutation when
  unsharding).  Edges are bucketed per (block, src-half) -- dma_gather
  indices are int16, so the 50000-row table is addressed as two halves --
  and padded to a multiple of 128 with src-index-0 dummies whose slot
  sentinel -1 gives all-zero H rows.  Per-dst inverse degrees are
  computed host-side (pure index data) and shipped as an input.
- Device kernel (identical SPMD program on all 8 cores), all-bf16 data
  path with fp32 PSUM accumulation:
  * one dma_gather per (dst-block, table-half) segment pulls that
    segment's bf16 source rows (256 B each) from HBM into SBUF.  Gathers
    are spread over all 4 SWDGE queues (the ucode routes queue q to Q7
    core pair q), so up to 4 calls generate descriptors concurrently.
  * per gather call, build the 0/1 edge->slot matrices H for all its
    groups in ONE DVE op via is_equal(slot_value, iota) with broadcast
    APs; padded edges give all-zero rows.
  * TensorE bf16 matmuls accumulate H^T @ G (feature sums) in PSUM per
    128-slot block.
  * per block: multiply by the precomputed 1/deg column, DMA the
    [128, 128] result tile to the output shard.
- Host gathers the 8 output shards into the full [50000, 128] output.
"""

import os
import sys

import numpy as np

for _p in ("/opt/trn_rl_repo",):
    if os.path.isdir(_p) and _p not in sys.path:
        sys.path.insert(0, _p)

from concourse import bacc, mybir  # noqa: E402
import concourse.bass as bass  # noqa: E402
import concourse.tile as tile  # noqa: E402
from concourse.bass_utils import run_bass_kernel_spmd  # noqa: E402

N_NODES = 50000
N_EDGES = 600000
D_FEAT = 128
N_CORES = 8
NLOC = N_NODES // N_CORES          # 6250 dst nodes per core
BLK = 128                          # dst slots per PSUM block
NB = (NLOC + BLK - 1) // BLK       # 49 blocks per core
HALF = 32768                       # int16 index limit for dma_gather
SWDGE_SCRATCH = 16384              # SWDGE descriptor ring: bytes/partition
NUM_QUEUES = 4                     # gather ucode: queue q -> Q7 core pair q
GBUFS = 10                         # gather/h tile pool depth
MAXG = 16                          # max groups per gather call (ring capacity)
NCHUNK = 4                         # idx table load split (startup overlap)
# -1 trailing padding is trimmed by the gather ucode, but with a static
# num_idxs_reg the decode-side ring bookkeeping (descs reserved from the reg
# value) desyncs from the Q7's actual pushed count -> device-wedging DMA
# corruption (observed on HW; CoreSim's reg==valid-count assert is the same
# protocol).  Keep False unless num_idxs_reg is a per-core runtime register.
PAD_NEG = False

_cache = {}


def _segments(g):
    """Call list: one dma_gather per (block, half) segment, split at MAXG.

    Returns list of dicts with keys: lst, b, g0 (group offset within its
    list), ncg, first (True for the first sub-call of the segment; the
    trailing -1 trim only applies to the last sub-call = segment tail).
    """
    aoff = np.concatenate([[0], np.cumsum(g[:, 0])])
    boff = np.concatenate([[0], np.cumsum(g[:, 1])])
    calls = []
    for b in range(g.shape[0]):
        for lst, off in ((0, aoff), (1, boff)):
            total = int(g[b, lst])
            s = 0
            while s < total:
                n = min(MAXG, total - s)
                calls.append(dict(lst=lst, b=b, g0=int(off[b]) + s, ncg=n,
                                  tail=(s + n == total)))
                s += n
    return calls


def _balance_core(dA, dB, gA, gB):
    """Assign one core's nodes to blocks, packing per-half edge counts under
    each block's group capacity.  Returns slot id per node, or None."""
    remA = gA.astype(np.int64) * BLK
    remB = gB.astype(np.int64) * BLK
    remN = np.full(NB, BLK, dtype=np.int64)
    order = np.argsort(-(dA * 2 + dB), kind="stable")
    blk_of = np.empty(dA.shape[0], np.int64)
    for n in order:
        ok = (remA >= dA[n]) & (remB >= dB[n]) & (remN > 0)
        if not ok.any():
            return None
        # maximize the tightest remaining margin (caps are lumpy, so best-fit
        # toward caps, not equal loads); node room as a light tiebreak
        mA = (remA - dA[n]) * 2
        mB = (remB - dB[n]) * 4
        score = np.where(ok, np.minimum(mA, mB) + remN * 8, -(10 ** 9))
        b = int(np.argmax(score))
        blk_of[n] = b
        remA[b] -= dA[n]
        remB[b] -= dB[n]
        remN[b] -= 1
    slot = np.empty(dA.shape[0], np.int64)
    for b in range(NB):
        nodes = np.where(blk_of == b)[0]
        slot[nodes] = b * BLK + np.arange(len(nodes))
    return slot


def _prepare(src, dst):
    """Inspector pass.

    The dst->slot mapping within each core is ours to choose, so a host-side
    bin-packing assigns nodes to 128-slot blocks such that every block's
    per-half edge count packs nearly exactly into 128-edge groups -- this
    removes the padding that a fixed dst-order layout pays (both the
    round-up per block and the max-over-cores slack).
    """
    core = dst // NLOC
    half = (src >= HALF).astype(np.int64)

    # per-node per-half degrees
    degA = np.bincount(dst[half == 0], minlength=N_NODES)
    degB = np.bincount(dst[half == 1], minlength=N_NODES)
    Acnt = degA.reshape(N_CORES, NLOC).sum(axis=1)
    Bcnt = degB.reshape(N_CORES, NLOC).sum(axis=1)

    GAL = int(np.ceil(Acnt.max() / BLK)) + 3
    GBL = int(np.ceil(Bcnt.max() / BLK)) + 3
    slots = None
    for _attempt in range(6):
        gA = np.full(NB, GAL // NB, dtype=np.int64)
        gA[:GAL % NB] += 1
        gB = np.full(NB, GBL // NB, dtype=np.int64)
        gB[:GBL % NB] += 1
        trial = []
        for c in range(N_CORES):
            s = _balance_core(degA[c * NLOC:(c + 1) * NLOC],
                              degB[c * NLOC:(c + 1) * NLOC], gA, gB)
            if s is None:
                break
            trial.append(s)
        if len(trial) == N_CORES:
            slots = trial
            break
        GAL += 2
        GBL += 2
    if slots is None:
        # fall back to the identity layout (node i -> slot i)
        slots = [np.arange(NLOC, dtype=np.int64) for _ in range(N_CORES)]

    # g from the REALIZED counts (feasible for balanced and fallback alike)
    slot_all = np.concatenate(slots)
    eb = slot_all[dst] // BLK
    cnt = np.zeros((N_CORES, NB, 2), dtype=np.int64)
    np.add.at(cnt, (core, eb, half), 1)
    g = (cnt + BLK - 1) // BLK
    g = g.max(axis=0)  # [NB, 2]
    g[(g.sum(axis=1) == 0), 0] = 1

    # per-edge block/srel under the balanced mapping
    # slots are per-core local; nodes are contiguous per core, so the
    # concatenation is indexable by global node id
    eslot = slot_all[dst]
    blk = eslot // BLK
    srel = (eslot % BLK).astype(np.float32)

    aoff = np.concatenate([[0], np.cumsum(g[:, 0])])
    boff = np.concatenate([[0], np.cumsum(g[:, 1])])
    GA, GB = int(aoff[-1]), int(boff[-1])
    G = GA + GB

    calls = _segments(g)
    neg_ok = np.zeros((NB, 2), dtype=bool)
    for c in calls:
        if c["tail"] and PAD_NEG:
            neg_ok[c["b"], c["lst"]] = True

    key = ((core * NB + blk) * 2 + half)
    order = np.argsort(key, kind="stable")
    key_sorted = key[order]
    src_sorted = src[order]
    srel_sorted = srel[order]
    seg_starts = np.searchsorted(key_sorted, np.arange(N_CORES * NB * 2))
    seg_ends = np.searchsorted(key_sorted, np.arange(N_CORES * NB * 2), side="right")

    idx_vals = np.zeros((N_CORES, G, 128), dtype=np.int16)
    slot_vals = np.full((N_CORES, G, 128), -1.0, dtype=np.float32)
    for c in range(N_CORES):
        for b in range(NB):
            for h in range(2):
                s, e = seg_starts[(c * NB + b) * 2 + h], seg_ends[(c * NB + b) * 2 + h]
                n = e - s
                ng = int(g[b, h])
                if ng == 0:
                    continue
                assert n <= ng * 128
                g0 = (aoff[b] if h == 0 else GA + boff[b])
                iv = idx_vals[c, g0:g0 + ng].reshape(-1)
                sv = slot_vals[c, g0:g0 + ng].reshape(-1)
                if n:
                    sseg = src_sorted[s:e]
                    iv[:n] = (sseg - HALF * h).astype(np.int16)
                    sv[:n] = srel_sorted[s:e]
                if neg_ok[b, h]:
                    iv[n:] = -1

    # wrapped int16 layout for dma_gather: value (g, q) -> [q%16, 8*g + q//16],
    # replicated across the 8 sixteen-partition stripes
    w = idx_vals.reshape(N_CORES, G, 8, 16).transpose(0, 3, 1, 2).reshape(N_CORES, 16, G * 8)
    idxw = np.tile(w, (1, 8, 1))                       # [C, 128, G*8] int16
    slotw = slot_vals.transpose(0, 2, 1).copy()        # [C, 128, G] f32

    # per-core inverse degrees at their balanced slots, [C, 128, NB]
    deg = (degA + degB).astype(np.float64)
    invd = (1.0 / np.maximum(deg, 1.0)).astype(np.float32)
    invdw = np.zeros((N_CORES, 128, NB), dtype=np.float32)
    for c in range(N_CORES):
        col = np.zeros(NB * BLK, dtype=np.float32)
        col[slots[c]] = invd[c * NLOC:(c + 1) * NLOC]
        invdw[c] = col.reshape(NB, BLK).T

    layout = dict(g=g, aoff=aoff, boff=boff, GA=GA, GB=GB)
    return idxw, slotw, invdw, slots, layout


def _build_program(layout):
    g, aoff, boff = layout["g"], layout["aoff"], layout["boff"]
    GA, GB = layout["GA"], layout["GB"]
    G = GA + GB
    f32 = mybir.dt.float32
    bf16 = mybir.dt.bfloat16

    nc = bacc.Bacc("TRN2", target_bir_lowering=False, debug=False,
                   num_devices=N_CORES, dynamic_dma_scratch_size=SWDGE_SCRATCH,
                   num_swdge_queues=NUM_QUEUES)
    # two separate tensors: dma_gather's ucode mishandles nonzero source-AP
    # offsets on HW, so each int16-addressable half gets its own tensor
    embA = nc.dram_tensor("embA", [HALF, D_FEAT], bf16, kind="ExternalInput").ap()
    embB = nc.dram_tensor("embB", [N_NODES - HALF, D_FEAT], bf16, kind="ExternalInput").ap()
    # H is built with 1-input DVE ops (tensor_scalar is_equal against a
    # per-partition scalar): any 2-input DVE op streams via the second SBUF
    # read port, which is the port shared with GpSimd -- a lock that starves
    # SWDGE descriptor generation (the kernel's critical path).  bf16 at
    # 2 elem/cycle is exactly one port's width, so these ops never touch it.
    iota = nc.dram_tensor("iota", [128, BLK], bf16, kind="ExternalInput").ap()
    idxw = nc.dram_tensor("idxw", [128, G * 8], mybir.dt.int16, kind="ExternalInput").ap()
    slotw = nc.dram_tensor("slotw", [128, G], bf16, kind="ExternalInput").ap()
    invdw = nc.dram_tensor("invdw", [128, NB], f32, kind="ExternalInput").ap()
    out = nc.dram_tensor("out", [NB * BLK, D_FEAT], f32, kind="ExternalOutput").ap()

    calls = _segments(g)
    maxg = max(c["ncg"] for c in calls)
    # column ranges in idxw per call; bucket calls into NCHUNK idx tiles
    # (split at call boundaries) so early gathers start before the whole
    # index table has loaded
    cols = []
    for c in calls:
        scol0 = c["g0"] if c["lst"] == 0 else GA + c["g0"]
        cols.append((scol0 * 8, c["ncg"] * 8, scol0))
    total_cols = G * 8
    target = (total_cols + NCHUNK - 1) // NCHUNK
    first_cut = 128  # ~2 calls' worth of idx columns
    # calls are not column-ordered (A and B interleave); chunk by column space
    # instead: chunk k covers columns [k*target, (k+1)*target), and each call
    # is assigned to the chunk containing its first column; chunk tiles
    # overlap-load enough columns to cover calls that straddle a boundary.
    chunk_lo = [0] + [min(first_cut + k * target, total_cols) for k in range(NCHUNK - 1)]
    chunk_hi = [min(first_cut, total_cols)] + [min(first_cut + (k + 1) * target, total_cols) for k in range(NCHUNK - 1)]
    call_chunk = []
    for (c0, ncols, _s) in cols:
        if c0 < first_cut:
            k = 0
        else:
            k = min((c0 - first_cut) // target + 1, NCHUNK - 1)
        call_chunk.append(k)
        chunk_hi[k] = max(chunk_hi[k], c0 + ncols)

    with tile.TileContext(nc) as tc:
        with (
            tc.tile_pool(name="const", bufs=1) as cpool,
            tc.tile_pool(name="gath", bufs=GBUFS) as gpool,
            tc.tile_pool(name="hbuf", bufs=GBUFS) as hpool,
            tc.tile_pool(name="evict", bufs=6) as epool,
            tc.tile_pool(name="psum", bufs=6, space="PSUM") as ppool,
        ):
            idx_tiles = []
            for k in range(NCHUNK):
                w = chunk_hi[k] - chunk_lo[k]
                t = cpool.tile([128, w], mybir.dt.int16, tag=f"idx{k}")
                nc.sync.dma_start(out=t[:], in_=idxw[:, chunk_lo[k]:chunk_hi[k]])
                idx_tiles.append(t)
                if k == 0:
                    # small constants right after the first idx chunk
                    iota_sb = cpool.tile([128, BLK], bf16, tag="iota")
                    nc.sync.dma_start(out=iota_sb[:], in_=iota[:])
                    slot_sb = cpool.tile([128, G], bf16, tag="slot")
                    nc.sync.dma_start(out=slot_sb[:], in_=slotw[:])
                    invd_sb = cpool.tile([128, NB], f32, tag="invd")
                    nc.sync.dma_start(out=invd_sb[:], in_=invdw[:])

            srcs = {0: embA, 1: embB}
            # issue order: one gather + one h-build per call, block-major
            call_of = {}
            for k, c in enumerate(calls):
                call_of.setdefault((c["lst"], c["b"]), []).append(k)
            tiles = [None] * len(calls)

            def issue_call(k):
                c = calls[k]
                c0, ncols, scol0 = cols[k]
                ncg = c["ncg"]
                ck = call_chunk[k]
                it = idx_tiles[ck]
                t = gpool.tile([128, maxg * 128], bf16, tag="g")
                nc.gpsimd.dma_gather(
                    out_ap=t[:, :ncg * 128].rearrange("p (n e) -> p n e", e=128),
                    in_ap=srcs[c["lst"]],
                    idxs_ap=it[:, c0 - chunk_lo[ck]:c0 - chunk_lo[ck] + ncols],
                    num_idxs=ncg * 128,
                    num_idxs_reg=ncg * 128,
                    elem_size=D_FEAT,
                    single_packet=(ncg <= 8),
                )
                # batched H build for all ncg groups of this call: one DVE op.
                # (2-input DVE ops lock the GpSimd-shared SBUF port while they
                # stream, so keep them bf16 -- 2 elem/cycle halves the lock
                # time -- and batched: per-group ops cost ~700ns dispatch each
                # on HW, which swamps their 67ns of streaming.)
                h = hpool.tile([128, maxg * BLK], bf16, tag="h")
                sv = slot_sb[:, scol0:scol0 + ncg]
                sv3 = sv.unsqueeze(2).to_broadcast([128, ncg, BLK])
                io3 = iota_sb[:].unsqueeze(1).to_broadcast([128, ncg, BLK])
                nc.vector.tensor_tensor(
                    out=h[:, :ncg * BLK].rearrange("p (n e) -> p n e", e=BLK),
                    in0=sv3,
                    in1=io3,
                    op=mybir.AluOpType.is_equal,
                )
                tiles[k] = (t, h)

            for b in range(NB):
                bcalls = call_of.get((0, b), []) + call_of.get((1, b), [])
                for k in bcalls:
                    issue_call(k)
                psum_s = ppool.tile([128, BLK], f32, tag="ps")
                ngrp = sum(calls[k]["ncg"] for k in bcalls)
                i = 0
                for k in bcalls:
                    t, h = tiles[k]
                    for j in range(calls[k]["ncg"]):
                        nc.tensor.matmul(
                            out=psum_s[:],
                            lhsT=h[:, j * BLK:(j + 1) * BLK],
                            rhs=t[:, j * 128:(j + 1) * 128],
                            start=(i == 0), stop=(i == ngrp - 1),
                        )
                        i += 1
                    tiles[k] = None
                ot = epool.tile([128, BLK], f32, tag="ot")
                nc.scalar.activation(
                    out=ot[:], in_=psum_s[:],
                    func=mybir.ActivationFunctionType.Copy,
                    scale=invd_sb[:, b:b + 1],
                )
                nc.sync.dma_start(out=out[b * BLK:(b + 1) * BLK, :],
                                  in_=ot[:, :])

    # Tile's scheduling pass reorders instructions and round-robins SWDGE
    # completion sems over 8 DMASW lanes in FINAL order.  A sem may only ever
    # be incremented from one SWDGE queue (ring-reclaim correctness), so the
    # queue must be a function of the assigned lane: queue = lane % NUM_QUEUES.
    if NUM_QUEUES > 1:
        from concourse.tile_scheduler import PROC_NAME_TO_IDX
        lane_of = {PROC_NAME_TO_IDX[f"DMASW{i}"]: i for i in range(8)}
        fn = nc.m.functions[0]
        insts = [i for blk_ in fn.blocks for i in blk_.instructions]
        for inst in insts:
            if isinstance(inst, mybir.InstDMAGatherAnt):
                lane = lane_of.get(inst.bass_scheduled_proc)
                assert lane is not None, "gather not on a DMASW lane"
                # lane // 2, not lane % 4: calls alternate big-A / small-B, so
                # modulo would put every big A call on queues 0,2 and bound
                # throughput at 2 pairs; pairing adjacent lanes gives each
                # queue one A-lane and one B-lane
                inst.queue_num = (lane // 2) % NUM_QUEUES

    nc.compile()
    return nc


def _in_maps(author_emb, src, dst):
    emb = np.ascontiguousarray(np.asarray(author_emb, dtype=np.float32))
    src = np.asarray(src).astype(np.int64)
    dst = np.asarray(dst).astype(np.int64)
    assert emb.shape == (N_NODES, D_FEAT) and src.shape == (N_EDGES,)

    idxw, slotw, invdw, slots, layout = _prepare(src, dst)
    key = (layout["GA"], layout["GB"], layout["g"].tobytes())
    if key not in _cache:
        _cache[key] = _build_program(layout)
    nc = _cache[key]

    import ml_dtypes
    embh = emb.astype(ml_dtypes.bfloat16)
    iota_np = np.broadcast_to(np.arange(BLK, dtype=np.float32), (128, BLK)).astype(ml_dtypes.bfloat16)
    embA = np.ascontiguousarray(embh[:HALF])
    embB = np.ascontiguousarray(embh[HALF:])
    maps = [
        {"embA": embA, "embB": embB, "iota": np.ascontiguousarray(iota_np),
         "idxw": idxw[c], "slotw": slotw[c].astype(ml_dtypes.bfloat16),
         "invdw": invdw[c]}
        for c in range(N_CORES)
    ]
    return nc, maps, slots


def kernel(author_emb, src, dst, n_nodes):
    nc, maps, slots = _in_maps(author_emb, src, dst)
    res = run_bass_kernel_spmd(nc, maps, list(range(N_CORES)))
    out = np.empty((N_NODES, D_FEAT), dtype=np.float32)
    for c in range(N_CORES):
        out[c * NLOC:(c + 1) * NLOC] = res.results[c]["out"][slots[c]]
    return out
